# revision 17
# baseline (speedup 1.0000x reference)
"""Self-contained Trainium2 Bass kernel for MegatronQwenAttention.

Sharding: sequence-parallel over 8 cores. Core c owns query rows
[c*256, (c+1)*256). Per core: QKV projections for its rows (full weights
streamed), RoPE on-chip (PE permute + DVE), AllGather of kT/v across
cores, full GQA attention for its rows (transposed-scores layout),
o_proj for its rows. All matmuls in fp32r.
"""

import sys
import numpy as np

if "/opt/trn_rl_repo" not in sys.path:
    sys.path.insert(0, "/opt/trn_rl_repo")

SEQ = 2048
HID = 3584
NH = 28          # query heads
NKV = 4          # kv heads
HD = 128         # head dim
KV = 512         # kv proj width
NC_ = 8          # cores
SLC = SEQ // NC_  # 256 rows per core
NK = HID // 128  # 28 contraction chunks
NKH = NK // 2    # 14 chunks per half
ROPE_THETA = 1000000.0

_compiled = {}
_last_in_maps = None


def _build():
    import concourse.bass as bass  # noqa: F401
    import concourse.mybir as mybir
    import concourse.tile as tile
    from concourse import bacc
    from contextlib import ExitStack

    F32 = mybir.dt.float32
    F32R = mybir.dt.float32r
    AF = mybir.ActivationFunctionType
    ALU = mybir.AluOpType

    nc = bacc.Bacc("TRN2", target_bir_lowering=False, debug=False, num_devices=NC_)

    # ---- I/O -------------------------------------------------------------
    hsT = nc.dram_tensor("hsT", [HID, SLC], F32, kind="ExternalInput").ap()
    wq = nc.dram_tensor("wq", [HID, HID], F32, kind="ExternalInput").ap()
    wk = nc.dram_tensor("wk", [HID, KV], F32, kind="ExternalInput").ap()
    wv = nc.dram_tensor("wv", [HID, KV], F32, kind="ExternalInput").ap()
    wo = nc.dram_tensor("wo", [HID, HID], F32, kind="ExternalInput").ap()
    bq_r = nc.dram_tensor("bq_r", [1, HID], F32, kind="ExternalInput").ap()
    bk_r = nc.dram_tensor("bk_r", [1, KV], F32, kind="ExternalInput").ap()
    bv_r = nc.dram_tensor("bv_r", [1, KV], F32, kind="ExternalInput").ap()
    maskT = nc.dram_tensor("maskT", [SEQ, SLC], F32, kind="ExternalInput").ap()
    cosT2 = nc.dram_tensor("cosT2", [HD, SLC], F32, kind="ExternalInput").ap()
    sinT2 = nc.dram_tensor("sinT2", [HD, SLC], F32, kind="ExternalInput").ap()
    perm = nc.dram_tensor("perm", [HD, HD], F32, kind="ExternalInput").ap()
    ident = nc.dram_tensor("ident", [HD, HD], F32, kind="ExternalInput").ap()
    ones_in = nc.dram_tensor("ones_in", [SLC, 1], F32, kind="ExternalInput").ap()

    out_part = nc.dram_tensor("out_part", [SLC, HID], F32, kind="ExternalOutput").ap()
    kT_part = nc.dram_tensor("kT_part", [KV, SLC], F32, kind="ExternalOutput").ap()
    v_part = nc.dram_tensor("v_part", [SLC, KV], F32, kind="ExternalOutput").ap()

    with tile.TileContext(nc) as tc, ExitStack() as top:
        const = top.enter_context(tc.tile_pool(name="const", bufs=1))
        dram = top.enter_context(tc.tile_pool(name="dram", bufs=1, space="DRAM"))

        ones_col = const.tile([128, 1], F32R, name="ones_col")
        nc.sync.dma_start(out=ones_col[:], in_=ones_in[0:128, :].bitcast(F32R))
        ones_row = const.tile([1, SLC], F32R, name="ones_row")
        nc.sync.dma_start(
            out=ones_row[:],
            in_=ones_in.rearrange("(one p) x -> one (p x)", one=1).bitcast(F32R),
        )
        perm_sb = const.tile([HD, HD], F32R, name="perm_sb")
        nc.sync.dma_start(out=perm_sb[:], in_=perm[:].bitcast(F32R))
        ident_sb = const.tile([HD, HD], F32R, name="ident_sb")
        nc.sync.dma_start(out=ident_sb[:], in_=ident[:].bitcast(F32R))
        cosT2_sb = const.tile([HD, SLC], F32, name="cosT2_sb")
        nc.sync.dma_start(out=cosT2_sb[:], in_=cosT2[:])
        sinT2_sb = const.tile([HD, SLC], F32, name="sinT2_sb")
        nc.sync.dma_start(out=sinT2_sb[:], in_=sinT2[:])
        # mask resident: [128, 16, SLC]
        maskT_sb = const.tile([128, SEQ // 128, SLC], F32, name="maskT_sb")
        nc.sync.dma_start(
            out=maskT_sb[:], in_=maskT.rearrange("(nk p) s -> p nk s", p=128)
        )

        # AG bounce buffers
        kvT_bounce = dram.tile([KV, SLC], F32, name="kvT_bounce")
        v_bounce = dram.tile([SLC, KV], F32, name="v_bounce")
        kT_ag = dram.tile([NC_ * KV, SLC], F32, name="kT_ag")
        v_ag = dram.tile([SEQ, KV], F32, name="v_ag")

        qT_pool = top.enter_context(tc.tile_pool(name="qT", bufs=1))
        qT_sb = qT_pool.tile([128, NH, SLC], F32R, name="qT_sb")

        with ExitStack() as proj:
            pconst = proj.enter_context(tc.tile_pool(name="pconst", bufs=1))
            hsT_sb = pconst.tile([128, NK, SLC], F32R, name="hsT_sb")
            nc.sync.dma_start(
                out=hsT_sb[:],
                in_=hsT.rearrange("(nk p) s -> p nk s", p=128).bitcast(F32R),
            )
            bq_sb = pconst.tile([1, HID], F32R, name="bq_sb")
            nc.sync.dma_start(out=bq_sb[:], in_=bq_r[:].bitcast(F32R))
            bk_sb = pconst.tile([1, KV], F32R, name="bk_sb")
            nc.sync.dma_start(out=bk_sb[:], in_=bk_r[:].bitcast(F32R))
            bv_sb = pconst.tile([1, KV], F32R, name="bv_sb")
            nc.sync.dma_start(out=bv_sb[:], in_=bv_r[:].bitcast(F32R))

            # ---- Phase 1: k/v projections + AG ---------------------------
            with ExitStack() as ph1:
                wkv_pool = ph1.enter_context(tc.tile_pool(name="wkv", bufs=1))
                kv_ps = ph1.enter_context(
                    tc.tile_pool(name="kv_ps", bufs=1, space="PSUM")
                )
                rp_ps = ph1.enter_context(
                    tc.tile_pool(name="rp_ps", bufs=2, space="PSUM")
                )
                kv_sb = ph1.enter_context(tc.tile_pool(name="kv_sb", bufs=2))

                kT_psl = [
                    kv_ps.tile([128, SLC], F32, name=f"kT_ps{j}")
                    for j in range(KV // 128)
                ]
                v_psl = [
                    kv_ps.tile([128, KV], F32, name=f"v_ps{i}")
                    for i in range(SLC // 128)
                ]
                for kg in range(2):
                    wk_h = wkv_pool.tile(
                        [128, NKH, KV], F32R, name=f"wk_h{kg}", tag="wk_h"
                    )
                    nc.sync.dma_start(
                        out=wk_h[:],
                        in_=wk[kg * NKH * 128 : (kg + 1) * NKH * 128, :]
                        .rearrange("(nk p) f -> p nk f", p=128)
                        .bitcast(F32R),
                    )
                    wv_h = wkv_pool.tile(
                        [128, NKH, KV], F32R, name=f"wv_h{kg}", tag="wv_h"
                    )
                    nc.sync.dma_start(
                        out=wv_h[:],
                        in_=wv[kg * NKH * 128 : (kg + 1) * NKH * 128, :]
                        .rearrange("(nk p) f -> p nk f", p=128)
                        .bitcast(F32R),
                    )
                    for j in range(KV // 128):
                        for kk in range(NKH):
                            nc.tensor.matmul(
                                kT_psl[j][:],
                                wk_h[:, kk, j * 128 : (j + 1) * 128],
                                hsT_sb[:, kg * NKH + kk, :],
                                start=(kg == 0 and kk == 0),
                                stop=False,
                                skip_group_check=True,
                            )
                    for i in range(SLC // 128):
                        for kk in range(NKH):
                            nc.tensor.matmul(
                                v_psl[i][:],
                                hsT_sb[:, kg * NKH + kk, i * 128 : (i + 1) * 128],
                                wv_h[:, kk, :],
                                start=(kg == 0 and kk == 0),
                                stop=False,
                                skip_group_check=True,
                            )

                for j in range(KV // 128):
                    nc.tensor.matmul(
                        kT_psl[j][:],
                        bk_sb[:, j * 128 : (j + 1) * 128],
                        ones_row[:],
                        start=False,
                        stop=True,
                        skip_group_check=True,
                    )
                    kT_raw = kv_sb.tile(
                        [128, SLC], F32R, name=f"kT_raw{j}", tag="kT_raw"
                    )
                    nc.scalar.activation(kT_raw[:], kT_psl[j][:], AF.Copy)
                    # rope (transposed layout)
                    shift_ps = rp_ps.tile(
                        [128, SLC], F32, name=f"kshift{j}", tag="kshift"
                    )
                    nc.tensor.matmul(
                        shift_ps[:], perm_sb[:], kT_raw[:], start=True, stop=True
                    )
                    t1 = kv_sb.tile([128, SLC], F32, name=f"kt1_{j}", tag="kt1")
                    nc.vector.tensor_tensor(
                        t1[:], kT_raw[:].bitcast(F32), cosT2_sb[:], ALU.mult
                    )
                    t2 = kv_sb.tile([128, SLC], F32, name=f"kt2_{j}", tag="kt2")
                    nc.vector.tensor_tensor(t2[:], shift_ps[:], sinT2_sb[:], ALU.mult)
                    kT_roped = kv_sb.tile(
                        [128, SLC], F32, name=f"kT_roped{j}", tag="kT_roped"
                    )
                    nc.vector.tensor_tensor(kT_roped[:], t1[:], t2[:], ALU.add)
                    nc.sync.dma_start(
                        out=kvT_bounce[j * 128 : (j + 1) * 128, :], in_=kT_roped[:]
                    )
                    nc.sync.dma_start(
                        out=kT_part[j * 128 : (j + 1) * 128, :], in_=kT_roped[:]
                    )

                for i in range(SLC // 128):
                    nc.tensor.matmul(
                        v_psl[i][:],
                        ones_row[:, :128],
                        bv_sb[:],
                        start=False,
                        stop=True,
                        skip_group_check=True,
                    )
                    v_sb = kv_sb.tile([128, KV], F32, name=f"v_sb{i}", tag="v_sb")
                    nc.vector.tensor_copy(v_sb[:], v_psl[i][:])
                    nc.sync.dma_start(
                        out=v_bounce[i * 128 : (i + 1) * 128, :], in_=v_sb[:]
                    )
                    nc.sync.dma_start(
                        out=v_part[i * 128 : (i + 1) * 128, :], in_=v_sb[:]
                    )

                nc.gpsimd.collective_compute(
                    "AllGather", mybir.AluOpType.bypass,
                    replica_groups=[list(range(NC_))],
                    ins=[kvT_bounce[:].opt()], outs=[kT_ag[:].opt()],
                )
                nc.gpsimd.collective_compute(
                    "AllGather", mybir.AluOpType.bypass,
                    replica_groups=[list(range(NC_))],
                    ins=[v_bounce[:].opt()], outs=[v_ag[:].opt()],
                )

            # ---- Phase 2: q projection + rope + transpose ----------------
            with ExitStack() as ph2:
                wq_pool = ph2.enter_context(tc.tile_pool(name="wq_pool", bufs=2))
                q_ps = ph2.enter_context(tc.tile_pool(name="q_ps", bufs=1, space="PSUM"))
                qt_ps = ph2.enter_context(
                    tc.tile_pool(name="qt_ps", bufs=2, space="PSUM")
                )
                q_sb = ph2.enter_context(tc.tile_pool(name="q_sb", bufs=3))

                for n in range(HID // 512):  # 7 column slices
                    qn_psl = [
                        q_ps.tile([128, 512], F32, name=f"qn_ps{n}_{i}", tag=f"qn_ps{i}")
                        for i in range(SLC // 128)
                    ]
                    for kg in range(2):
                        wq_sl = wq_pool.tile(
                            [128, NKH, 512], F32R, name=f"wq_sl{n}_{kg}", tag="wq_sl"
                        )
                        nc.sync.dma_start(
                            out=wq_sl[:],
                            in_=wq[
                                kg * NKH * 128 : (kg + 1) * NKH * 128,
                                n * 512 : (n + 1) * 512,
                            ]
                            .rearrange("(nk p) f -> p nk f", p=128)
                            .bitcast(F32R),
                        )
                        for i in range(SLC // 128):
                            for kk in range(NKH):
                                nc.tensor.matmul(
                                    qn_psl[i][:],
                                    hsT_sb[:, kg * NKH + kk, i * 128 : (i + 1) * 128],
                                    wq_sl[:, kk, :],
                                    start=(kg == 0 and kk == 0),
                                    stop=False,
                                    skip_group_check=True,
                                )
                    for i in range(SLC // 128):
                        nc.tensor.matmul(
                            qn_psl[i][:],
                            ones_row[:, :128],
                            bq_sb[:, n * 512 : (n + 1) * 512],
                            start=False,
                            stop=True,
                            skip_group_check=True,
                        )
                        q_nat = q_sb.tile(
                            [128, 512], F32R, name=f"q_nat{n}_{i}", tag="q_nat"
                        )
                        nc.scalar.activation(q_nat[:], qn_psl[i][:], AF.Copy)
                        sl = slice(i * 128, (i + 1) * 128)
                        for hh in range(4):
                            h = n * 4 + hh
                            tr_ps = qt_ps.tile(
                                [128, 128], F32R, name=f"tr_ps{h}_{i}", tag="tr_ps"
                            )
                            nc.tensor.transpose(
                                tr_ps[:], q_nat[:, hh * 128 : (hh + 1) * 128],
                                ident_sb[:],
                            )
                            qT_raw = q_sb.tile(
                                [128, 128], F32R, name=f"qT_raw{h}_{i}", tag="qT_raw"
                            )
                            nc.vector.tensor_copy(qT_raw[:], tr_ps[:])
                            shift_ps = qt_ps.tile(
                                [128, 128], F32, name=f"qshift{h}_{i}", tag="qshift"
                            )
                            nc.tensor.matmul(
                                shift_ps[:], perm_sb[:], qT_raw[:],
                                start=True, stop=True,
                            )
                            t1 = q_sb.tile(
                                [128, 128], F32, name=f"qt1_{h}_{i}", tag="qt1"
                            )
                            nc.vector.tensor_tensor(
                                t1[:], qT_raw[:].bitcast(F32), cosT2_sb[:, sl], ALU.mult
                            )
                            t2 = q_sb.tile(
                                [128, 128], F32, name=f"qt2_{h}_{i}", tag="qt2"
                            )
                            nc.vector.tensor_tensor(
                                t2[:], shift_ps[:], sinT2_sb[:, sl], ALU.mult
                            )
                            nc.vector.tensor_tensor(
                                qT_sb[:, h, sl], t1[:], t2[:], ALU.add
                            )

        # ---- Phase 3: attention ------------------------------------------
        NKC = SEQ // 128  # 16 key chunks
        attn_pool = top.enter_context(tc.tile_pool(name="attnT", bufs=1))
        attnT_sb = attn_pool.tile([128, NH, SLC], F32R, name="attnT_sb")

        with ExitStack() as ph3:
            kv_glob = ph3.enter_context(tc.tile_pool(name="kv_glob", bufs=1))
            sc_ps = ph3.enter_context(tc.tile_pool(name="sc_ps", bufs=2, space="PSUM"))
            av_ps = ph3.enter_context(tc.tile_pool(name="av_ps", bufs=2, space="PSUM"))
            sum_ps = ph3.enter_context(
                tc.tile_pool(name="sum_ps", bufs=2, space="PSUM")
            )
            at_sb = ph3.enter_context(tc.tile_pool(name="at_sb", bufs=4))

            kT_g_sb = []
            v_g_sb = []
            for g in range(NKV):
                # kT_ag row index = r*KV + g*128 + p (core r, kv head g, dim p)
                kt = kv_glob.tile([128, NC_, 1, SLC], F32R, name=f"kT_g{g}")
                nc.sync.dma_start(
                    out=kt[:],
                    in_=kT_ag.rearrange("(r j p) s -> p r j s", r=NC_, p=128)[
                        :, :, g : g + 1, :
                    ].bitcast(F32R),
                )
                kT_g_sb.append(kt)
                vt = kv_glob.tile([128, NKC, 128], F32R, name=f"v_g{g}")
                nc.sync.dma_start(
                    out=vt[:],
                    in_=v_ag[:, g * 128 : (g + 1) * 128]
                    .rearrange("(c p) d -> p c d", p=128)
                    .bitcast(F32R),
                )
                v_g_sb.append(vt)

            for h in range(NH):
                g = h // (NH // NKV)
                kt = kT_g_sb[g]
                vt = v_g_sb[g]
                av = av_ps.tile([128, SLC], F32, name=f"av{h}", tag="av")
                sm = sum_ps.tile([1, SLC], F32, name=f"sm{h}", tag="sm")
                for c in range(NKC):
                    # global key chunk c -> core r = c//2, local half = c%2
                    r, half = c // 2, c % 2
                    k_chunk = kt[:, r, 0, half * 128 : (half + 1) * 128]
                    sc = sc_ps.tile([128, SLC], F32, name=f"sc{h}_{c}", tag="sc")
                    nc.tensor.matmul(
                        sc[:], k_chunk, qT_sb[:, h, :], start=True, stop=True
                    )
                    nc.vector.tensor_tensor(sc[:], sc[:], maskT_sb[:, c, :], ALU.add)
                    ex = at_sb.tile([128, SLC], F32R, name=f"ex{h}_{c}", tag="ex")
                    nc.scalar.activation(ex[:], sc[:], AF.Exp)
                    nc.tensor.matmul(
                        sm[:], ones_col[:], ex[:],
                        start=(c == 0), stop=(c == NKC - 1), skip_group_check=True,
                    )
                    nc.tensor.matmul(
                        av[:], vt[:, c, :], ex[:],
                        start=(c == 0), stop=(c == NKC - 1), skip_group_check=True,
                    )
                recip = at_sb.tile([1, SLC], F32R, name=f"recip{h}", tag="recip")
                with nc.allow_low_precision(reason="f32r is f32 bits"):
                    nc.vector.reciprocal(recip[:], sm[:])
                rb_ps = sum_ps.tile([128, SLC], F32, name=f"rb_ps{h}", tag="rb_ps")
                nc.tensor.matmul(
                    rb_ps[:], ones_row[:, :128], recip[:], start=True, stop=True
                )
                rb_sb = at_sb.tile([128, SLC], F32, name=f"rb_sb{h}", tag="rb_sb")
                nc.vector.tensor_copy(rb_sb[:], rb_ps[:])
                nc.vector.tensor_tensor(attnT_sb[:, h, :], av[:], rb_sb[:], ALU.mult)

        # ---- Phase 4: o_proj ---------------------------------------------
        with ExitStack() as ph4:
            wo_pool = ph4.enter_context(tc.tile_pool(name="wo_pool", bufs=2))
            o_ps = ph4.enter_context(tc.tile_pool(name="o_ps", bufs=1, space="PSUM"))
            o_sb = ph4.enter_context(tc.tile_pool(name="o_sb", bufs=3))

            for n in range(HID // 256):  # 14 column slices of wo
                on_psl = [
                    o_ps.tile([128, 256], F32, name=f"on_ps{n}_{i}", tag=f"on_ps{i}")
                    for i in range(SLC // 128)
                ]
                for kg in range(2):
                    wo_sl = wo_pool.tile(
                        [128, NKH, 256], F32R, name=f"wo_sl{n}_{kg}", tag="wo_sl"
                    )
                    nc.sync.dma_start(
                        out=wo_sl[:],
                        in_=wo[
                            kg * NKH * 128 : (kg + 1) * NKH * 128,
                            n * 256 : (n + 1) * 256,
                        ]
                        .rearrange("(nk p) f -> p nk f", p=128)
                        .bitcast(F32R),
                    )
                    for i in range(SLC // 128):
                        for kk in range(NKH):
                            nc.tensor.matmul(
                                on_psl[i][:],
                                attnT_sb[:, kg * NKH + kk, i * 128 : (i + 1) * 128],
                                wo_sl[:, kk, :],
                                start=(kg == 0 and kk == 0),
                                stop=(kg == 1 and kk == NKH - 1),
                                skip_group_check=True,
                            )
                for i in range(SLC // 128):
                    on_sb = o_sb.tile(
                        [128, 256], F32, name=f"on_sb{n}_{i}", tag="on_sb"
                    )
                    nc.scalar.activation(on_sb[:], on_psl[i][:], AF.Copy)
                    nc.sync.dma_start(
                        out=out_part[
                            i * 128 : (i + 1) * 128, n * 256 : (n + 1) * 256
                        ],
                        in_=on_sb[:],
                    )

    nc.compile()
    return nc


def kernel(hidden_states, attention_mask, position_ids, wq, bq, wk, bk, wv, bv, wo):
    from concourse.bass_utils import run_bass_kernel_spmd

    if "nc" not in _compiled:
        _compiled["nc"] = _build()
    nc = _compiled["nc"]

    hs = np.asarray(hidden_states, np.float32)[0]          # [SEQ, HID]
    mask = np.asarray(attention_mask, np.float32)[0, 0]    # [SEQ, SEQ]
    pos = np.asarray(position_ids)[0].astype(np.float32)   # [SEQ]
    wq = np.asarray(wq, np.float32)
    wk = np.asarray(wk, np.float32)
    wv = np.asarray(wv, np.float32)
    wo = np.asarray(wo, np.float32)
    bq = np.asarray(bq, np.float32)
    bk = np.asarray(bk, np.float32)
    bv = np.asarray(bv, np.float32)

    scale = 1.0 / np.sqrt(np.float32(HD))
    wq_s = (wq * scale).astype(np.float32)
    bq_s = (bq * scale).astype(np.float32)

    hsT = np.ascontiguousarray(hs.T)                       # [HID, SEQ]
    maskT = np.ascontiguousarray(mask.T)                   # [key, row]

    dim = HD // 2
    freqs = (1.0 / ROPE_THETA ** (np.arange(0, dim, dtype=np.float32) / dim)).astype(
        np.float32
    )
    t = pos[:, None] * freqs[None, :]                      # [SEQ, 64]
    cos_t = np.cos(t).astype(np.float32)
    sin_t = np.sin(t).astype(np.float32)
    # transposed rope tables [dim 128, SEQ]: cos2[d] = cos[d % 64],
    # sin2 signed: -sin for d<64, +sin for d>=64
    cosT2 = np.concatenate([cos_t.T, cos_t.T], axis=0)     # [128, SEQ]
    sinT2 = np.concatenate([-sin_t.T, sin_t.T], axis=0)    # [128, SEQ]

    P = np.zeros((HD, HD), np.float32)
    P[(np.arange(HD) + 64) % HD, np.arange(HD)] = 1.0      # out[m]=in[(m+64)%128]
    ident = np.eye(HD, dtype=np.float32)
    ones = np.ones((SLC, 1), np.float32)

    in_maps = []
    for c in range(NC_):
        rows = slice(c * SLC, (c + 1) * SLC)
        in_maps.append(
            {
                "hsT": np.ascontiguousarray(hsT[:, rows]),
                "wq": wq_s,
                "wk": wk,
                "wv": wv,
                "wo": wo,
                "bq_r": bq_s.reshape(1, HID),
                "bk_r": bk.reshape(1, KV),
                "bv_r": bv.reshape(1, KV),
                "maskT": np.ascontiguousarray(maskT[:, rows]),
                "cosT2": np.ascontiguousarray(cosT2[:, rows]),
                "sinT2": np.ascontiguousarray(sinT2[:, rows]),
                "perm": P,
                "ident": ident,
                "ones_in": ones,
            }
        )

    global _last_in_maps
    _last_in_maps = in_maps
    res = run_bass_kernel_spmd(nc, in_maps, core_ids=list(range(NC_)))

    out = np.empty((1, SEQ, HID), np.float32)
    cache_k = np.empty((1, SEQ, NKV, HD), np.float32)
    cache_v = np.empty((1, SEQ, NKV, HD), np.float32)
    for c in range(NC_):
        rows = slice(c * SLC, (c + 1) * SLC)
        r = res.results[c]
        out[0, rows, :] = r["out_part"]
        # kT_part [KV, SLC] -> [SLC, NKV, HD]
        cache_k[0, rows] = r["kT_part"].reshape(NKV, HD, SLC).transpose(2, 0, 1)
        cache_v[0, rows] = r["v_part"].reshape(SLC, NKV, HD)
    return out, cache_k, cache_v
